# revision 1
# baseline (speedup 1.0000x reference)
"""Trainium2 Bass kernel for nn_BDH_1726576853700 (sparse_attention).

3-layer sparse-attention net: B=1, T=1024, D=256, NH=4, N=8192, VOCAB=256.

Sharding over 8 NeuronCores: device d -> (head h=d//2, half=d%2) — each device
owns a 4096-wide slice of one head's sparse latent dim.  Within the slice the
latent index is permuted evens-first so the RoPE pair partner sits exactly 2048
rows away (tile i <-> tile i+16), turning the pair rotation into whole-tile
elementwise ops.  Per layer:
  - x_sparse^T = relu(enc_w^T @ x^T)   (local)
  - qr = rope(x_sparse)                (local, host-precomputed cos/sin tables)
  - S_partial = qr^T qr (local n contraction), strictly-causal masked
  - ykv_partial = S_masked^T @ x ; pair AllReduce (the two halves of one head)
  - ykv_ln = layernorm(ykv); y_sparse^T = relu(encv_w^T @ ykv_ln^T) (local)
  - ymlp^T_partial = dec^T-contracted with (x_sparse * y_sparse)    (local)
  - 8-way AllReduce(ymlp); x = ln(x + ln(ymlp)) (replicated)
Collectives run in fp16 (halves wire bytes); matmuls run in fp16 with fp32
PSUM accumulation; the residual stream x is kept in fp32 on-chip.

PSUM discipline: every accumulation group owns its bank(s) exclusively —
`start=True` clears has_written bits for the WHOLE bank, so two interleaved
groups must never share a bank.
"""

import math
import sys

for _p in ("/opt/trn_rl_repo",):
    if _p not in sys.path:
        sys.path.insert(0, _p)

import numpy as np

import concourse.bass as bass
import concourse.mybir as mybir
import concourse.tile as tile
from concourse import bacc, bass_utils
from concourse.masks import make_identity

# ---- problem constants (hardcoded per contract) ----
B, T, D, NH, N = 1, 1024, 256, 4, 8192
VOCAB = 256
N_LAYER = 3
EPS = 1e-5
TWO_PI = 2.0 * math.pi
N_CORES = 8
NLOC = N // 2          # latent columns per device: 4096
P = 128
NT = T // P            # 8 t-tiles
KD = D // P            # 2 d-tiles
NM = NLOC // P         # 32 n-tiles per device
NPAIR = NM // 2        # 16 rope pairs
HDT = mybir.dt.float16     # on-chip activation dtype
F32 = mybir.dt.float32
YKV_SCALE = 1.0 / 256.0    # keeps ykv in fp16 range; LN downstream is
                           # scale-invariant so the result is unchanged

_CACHE = {}


def _build_program(dbg=False, use_collectives=True, rope_gpsimd=False, skip_scores=False, skip_proj=False, n_layers=N_LAYER):
    def emit_allreduce(nc, groups, ins, outs):
        if use_collectives:
            nc.gpsimd.collective_compute(
                "AllReduce", mybir.AluOpType.add, replica_groups=groups,
                ins=ins, outs=outs)
        else:
            # timing/sim variant: replace the collective with a plain copy
            nc.sync.dma_start(outs[0], ins[0])
    nc = bacc.Bacc("TRN2", target_bir_lowering=False, debug=False,
                   num_devices=N_CORES)
    dbg_tensors = {}
    if dbg:
        for nm, shape, dt in [
            ("dbg_x0ln", [T, D], F32),
            ("dbg_xsp", [NM * P, T], HDT),
            ("dbg_qr", [NM * P, T], HDT),
            ("dbg_ykvpre", [T, D], HDT),
            ("dbg_ykvpost", [T, D], HDT),
            ("dbg_ykvT", [D, T], HDT),
            ("dbg_ymlppre", [D, T], HDT),
            ("dbg_ymlppost", [D, T], HDT),
            ("dbg_x1", [T, D], F32),
        ]:
            dbg_tensors[nm] = nc.dram_tensor(nm, shape, dt,
                                             kind="ExternalOutput")

    x0_d = nc.dram_tensor("x0", [T, D], F32, kind="ExternalInput")
    encw_d = nc.dram_tensor("encw", [D, NLOC], HDT, kind="ExternalInput")
    encvw_d = nc.dram_tensor("encvw", [D, NLOC], HDT, kind="ExternalInput")
    decw_d = nc.dram_tensor("decw", [NLOC, D], HDT, kind="ExternalInput")
    ct_d = nc.dram_tensor("ct", [NLOC // 2, T], HDT, kind="ExternalInput")
    st_d = nc.dram_tensor("st", [NLOC // 2, T], HDT, kind="ExternalInput")
    lmh_d = nc.dram_tensor("lmh", [D, VOCAB], HDT, kind="ExternalInput")
    umask_d = nc.dram_tensor("umask", [P, P], F32, kind="ExternalInput")
    logits_d = nc.dram_tensor("logits", [T, VOCAB], F32, kind="ExternalOutput")

    PAIR_GROUPS = [[0, 1], [2, 3], [4, 5], [6, 7]]
    ALL_GROUP = [list(range(N_CORES))]

    with tile.TileContext(nc) as tc:
        persist = tc.alloc_tile_pool(name="persist", bufs=1)
        dram = tc.alloc_tile_pool(name="dram", bufs=1, space="DRAM")

        # persistent SBUF state
        x_sp = persist.tile([P, NM, T], HDT)        # x_sparse^T tiles
        qr = persist.tile([P, NM, T], HDT)          # roped x_sparse^T
        x_f32 = persist.tile([P, NT, D], F32)       # residual stream (natural)
        x_h = persist.tile([P, NT, D], HDT)         # x natural fp16
        xT_h = persist.tile([P, KD, T], HDT)        # x^T fp16
        ykvT_h = persist.tile([P, KD, T], HDT)      # ykv_ln^T fp16
        lmh_sb = persist.tile([P, KD, VOCAB], HDT)
        umask_sb = persist.tile([P, P], F32)
        ident = persist.tile([P, P], HDT)

        eps_sb = persist.tile([P, 1], F32)
        nc.vector.memset(eps_sb[:], float(EPS))
        nc.sync.dma_start(umask_sb[:], umask_d.ap())
        make_identity(nc, ident[:])
        for k in range(KD):
            nc.sync.dma_start(lmh_sb[:, k, :], lmh_d.ap()[k * P:(k + 1) * P, :])

        # streaming / working pools (live across the whole kernel)
        wenc = tc.alloc_tile_pool(name="wenc", bufs=3)
        wdec = tc.alloc_tile_pool(name="wdec", bufs=4)
        csp = tc.alloc_tile_pool(name="csp", bufs=2)
        ropep = tc.alloc_tile_pool(name="ropep", bufs=2)
        schp = tc.alloc_tile_pool(name="schp", bufs=2)
        sdp = tc.alloc_tile_pool(name="sdp", bufs=2)
        yxp = tc.alloc_tile_pool(name="yxp", bufs=2)
        arp = tc.alloc_tile_pool(name="arp", bufs=1)
        lnp = tc.alloc_tile_pool(name="lnp", bufs=2)
        statp = tc.alloc_tile_pool(name="statp", bufs=4)

        def layer_norm(src_ap, out_ap):
            """LayerNorm over the free dim (size D) of a [P, D] tile."""
            stats = statp.tile([P, 6], F32, name="ln_stats")
            mv = statp.tile([P, 2], F32, name="ln_mv")
            rstd = statp.tile([P, 1], F32, name="ln_rstd")
            nc.vector.bn_stats(out=stats[:], in_=src_ap)
            nc.vector.bn_aggr(out=mv[:], in_=stats[:])
            nc.scalar.activation(out=rstd[:], in_=mv[:, 1:2],
                                 func=mybir.ActivationFunctionType.Sqrt,
                                 bias=eps_sb[:])
            nc.vector.reciprocal(out=rstd[:], in_=rstd[:])
            nc.vector.tensor_scalar(out=out_ap, in0=src_ap,
                                    scalar1=mv[:, 0:1], scalar2=rstd[:],
                                    op0=mybir.AluOpType.subtract,
                                    op1=mybir.AluOpType.mult)

        def transpose_into(dst_ap, src_ap, pst_pool):
            """PE-transpose a [P, P] fp16 SBUF block into dst (via PSUM)."""
            pst = pst_pool.tile([P, P], HDT, name="pst")
            nc.tensor.transpose(pst[:], src_ap, ident[:])
            nc.vector.tensor_copy(out=dst_ap, in_=pst[:])

        def set_x_from(j, src_f32_ap, pst_pool):
            """Write x_f32/x_h/xT_h for t-tile j from a normalized f32 tile."""
            if src_f32_ap is not x_f32:
                nc.vector.tensor_copy(out=x_f32[:, j, :], in_=src_f32_ap)
            nc.scalar.copy(out=x_h[:, j, :], in_=x_f32[:, j, :])
            for k in range(KD):
                transpose_into(xT_h[:, k, j * P:(j + 1) * P],
                               x_h[:, j, k * P:(k + 1) * P], pst_pool)

        # ---- initial x = ln(embed[idx]) (gather done on host into x0) ----
        with tc.tile_pool(name="ps_init", bufs=2, space="PSUM") as ps_init:
            for j in range(NT):
                x0t = lnp.tile([P, D], F32, name="x0t")
                nc.sync.dma_start(x0t[:], x0_d.ap()[j * P:(j + 1) * P, :])
                layer_norm(x0t[:], x_f32[:, j, :])
                set_x_from(j, x_f32, ps_init)
        if dbg:
            nc.sync.dma_start(
                dbg_tensors["dbg_x0ln"].ap().rearrange("(j p) d -> p j d", p=P),
                x_f32[:])

        # ---- layers ----
        for layer in range(n_layers):
            # Phase A: x_sparse^T = relu(enc^T x^T), then rope -> qr
            with tc.tile_pool(name=f"psA_{layer}", bufs=2,
                              space="PSUM") as psA:
                for m in range(NM):
                    ps = psA.tile([P, T], F32, name="psA")
                    et = wenc.tile([P, KD, P], HDT, name="enc_t")
                    nc.sync.dma_start(
                        et[:],
                        encw_d.ap().rearrange("(k p) n -> p k n", p=P)[
                            :, :, m * P:(m + 1) * P])
                    for c in range(2):
                        for k in range(1 if skip_proj else KD):
                            nc.tensor.matmul(
                                ps[:, c * 512:(c + 1) * 512],
                                lhsT=et[:, k, :],
                                rhs=xT_h[:, k, c * 512:(c + 1) * 512],
                                start=(k == 0),
                                stop=(k == (0 if skip_proj else KD - 1)))
                    nc.scalar.activation(
                        out=x_sp[:, m, :], in_=ps[:],
                        func=mybir.ActivationFunctionType.Relu)

                for i in range(NPAIR):
                    ctt = csp.tile([P, T], HDT, name="ctt")
                    stt = csp.tile([P, T], HDT, name="stt")
                    nc.sync.dma_start(ctt[:], ct_d.ap()[i * P:(i + 1) * P, :])
                    nc.sync.dma_start(stt[:], st_d.ap()[i * P:(i + 1) * P, :])
                    xe = x_sp[:, i, :]
                    xo = x_sp[:, i + NPAIR, :]
                    t1 = ropep.tile([P, T], HDT, name="rope_t1")
                    t2 = ropep.tile([P, T], HDT, name="rope_t2")
                    eng2 = nc.gpsimd if rope_gpsimd else nc.vector
                    nc.vector.tensor_mul(t1[:], xe, ctt[:])
                    eng2.tensor_mul(t2[:], xo, stt[:])
                    nc.vector.tensor_sub(qr[:, i, :], t1[:], t2[:])
                    t3 = ropep.tile([P, T], HDT, name="rope_t1")
                    t4 = ropep.tile([P, T], HDT, name="rope_t2")
                    nc.vector.tensor_mul(t3[:], xo, ctt[:])
                    eng2.tensor_mul(t4[:], xe, stt[:])
                    nc.vector.tensor_add(qr[:, i + NPAIR, :], t3[:], t4[:])

            if dbg and layer == 0:
                nc.sync.dma_start(
                    dbg_tensors["dbg_xsp"].ap().rearrange(
                        "(m p) t -> p m t", p=P), x_sp[:])
                nc.sync.dma_start(
                    dbg_tensors["dbg_qr"].ap().rearrange(
                        "(m p) t -> p m t", p=P), qr[:])

            # Phase B: S partial + causal mask + ykv partial accumulation.
            # c-major passes so the 4 live ykv accumulators each own a full
            # PSUM bank (plus 2 rotating banks for S chunks).
            ykv_pre = arp.tile([P, NT, D], HDT, name="ykv_pre")
            for c in range(2):
                with tc.tile_pool(name=f"psS_{layer}_{c}", bufs=3,
                                  space="PSUM") as psS, \
                     tc.tile_pool(name=f"psY_{layer}_{c}", bufs=1,
                                  space="PSUM") as psY:
                    ykv_ps = [psY.tile([P, D], F32, name=f"ykv_ps{j}",
                                       tag=f"ykv_ps{j}")
                              for j in range(4 * c, 4 * c + 4)]
                    for i in range(4 * c + 4):
                        # causal tiling: only columns t >= i*P are needed
                        base = max(c * 512, i * P)
                        width = (c + 1) * 512 - base
                        ps = psS.tile([P, 512], F32, name="psS")
                        for k in range(1 if skip_scores else NM):
                            nc.tensor.matmul(
                                ps[:, :width],
                                lhsT=qr[:, k, i * P:(i + 1) * P],
                                rhs=qr[:, k, base:base + width],
                                start=(k == 0),
                                stop=(k == (0 if skip_scores else NM - 1)))
                        sc = schp.tile([P, 512], HDT, name="schunk")
                        if i % 2 == 0:
                            nc.scalar.copy(out=sc[:, :width],
                                           in_=ps[:, :width])
                        else:
                            nc.vector.tensor_copy(out=sc[:, :width],
                                                  in_=ps[:, :width])
                        sd = None
                        if c == i // 4:
                            dcol = i * P - base
                            sd = sdp.tile([P, P], HDT, name="sdiag")
                            nc.vector.tensor_mul(sd[:],
                                                 ps[:, dcol:dcol + P],
                                                 umask_sb[:])
                        for j in range(max(4 * c, i), 4 * c + 4):
                            lhsT = sd[:] if j == i else \
                                sc[:, j * P - base:(j + 1) * P - base]
                            nc.tensor.matmul(
                                ykv_ps[j - 4 * c][:], lhsT=lhsT,
                                rhs=x_h[:, i, :],
                                start=(i == 0), stop=(i == j))
                    for j in range(4 * c, 4 * c + 4):
                        nc.scalar.mul(out=ykv_pre[:, j, :],
                                      in_=ykv_ps[j - 4 * c][:],
                                      mul=YKV_SCALE)

            if dbg and layer == 0:
                nc.sync.dma_start(
                    dbg_tensors["dbg_ykvpre"].ap().rearrange(
                        "(j p) d -> p j d", p=P), ykv_pre[:])

            # Phase C: pair AllReduce of ykv, layernorm, transpose
            ar_in = dram.tile([T, D], HDT, name=f"arin_{layer}",
                              tag=f"arin_{layer}")
            ar_out = dram.tile([T, D], HDT, name=f"arout_{layer}",
                               tag=f"arout_{layer}")
            nc.sync.dma_start(
                ar_in.rearrange("(j p) d -> p j d", p=P), ykv_pre[:])
            emit_allreduce(nc, PAIR_GROUPS, [ar_in.opt()], [ar_out.opt()])
            ykv_post = arp.tile([P, NT, D], HDT, name="ykv_post")
            nc.sync.dma_start(
                ykv_post[:], ar_out.rearrange("(j p) d -> p j d", p=P))
            with tc.tile_pool(name=f"psT_{layer}", bufs=2,
                              space="PSUM") as psT:
                for j in range(NT):
                    yl = lnp.tile([P, D], HDT, name="ykv_ln")
                    layer_norm(ykv_post[:, j, :], yl[:])
                    for k in range(KD):
                        transpose_into(ykvT_h[:, k, j * P:(j + 1) * P],
                                       yl[:, k * P:(k + 1) * P], psT)

            if dbg and layer == 0:
                nc.sync.dma_start(
                    dbg_tensors["dbg_ykvpost"].ap().rearrange(
                        "(j p) d -> p j d", p=P), ykv_post[:])
                nc.sync.dma_start(
                    dbg_tensors["dbg_ykvT"].ap().rearrange(
                        "(k p) t -> p k t", p=P), ykvT_h[:])

            # Phase D: y_sparse^T = relu(encv^T ykv_ln^T); xy = x_sp * y_sp;
            # ymlp^T accumulated transposed: lhsT = decoder tile, rhs = xy.
            # ymlp^T psum tiles span 2 banks each with exactly one
            # accumulation group per bank.
            ymlpT_pre = arp.tile([P, KD, T], HDT, name="ymlpT_pre")
            with tc.tile_pool(name=f"psD_{layer}", bufs=2,
                              space="PSUM") as psD, \
                 tc.tile_pool(name=f"psM_{layer}", bufs=1,
                              space="PSUM") as psM:
                ymlpT_ps = [psM.tile([P, T], F32, name=f"ymlpT_ps{k}",
                                     tag=f"ymlpT_ps{k}") for k in range(KD)]
                for m in range(NM):
                    ps = psD.tile([P, T], F32, name="psD")
                    et = wenc.tile([P, KD, P], HDT, name="encv_t")
                    nc.sync.dma_start(
                        et[:],
                        encvw_d.ap().rearrange("(k p) n -> p k n", p=P)[
                            :, :, m * P:(m + 1) * P])
                    for c in range(2):
                        for k in range(KD):
                            nc.tensor.matmul(
                                ps[:, c * 512:(c + 1) * 512],
                                lhsT=et[:, k, :],
                                rhs=ykvT_h[:, k, c * 512:(c + 1) * 512],
                                start=(k == 0), stop=(k == KD - 1))
                    ysp = yxp.tile([P, T], HDT, name="ysp")
                    nc.scalar.activation(
                        out=ysp[:], in_=ps[:],
                        func=mybir.ActivationFunctionType.Relu)
                    xy = yxp.tile([P, T], HDT, name="xy")
                    nc.vector.tensor_mul(xy[:], x_sp[:, m, :], ysp[:])
                    dm = wdec.tile([P, D], HDT, name="dec_t")
                    nc.sync.dma_start(dm[:],
                                      decw_d.ap()[m * P:(m + 1) * P, :])
                    for k in range(KD):
                        for c in range(2):
                            nc.tensor.matmul(
                                ymlpT_ps[k][:, c * 512:(c + 1) * 512],
                                lhsT=dm[:, k * P:(k + 1) * P],
                                rhs=xy[:, c * 512:(c + 1) * 512],
                                start=(m == 0), stop=(m == NM - 1))
                for k in range(KD):
                    nc.scalar.copy(out=ymlpT_pre[:, k, :],
                                   in_=ymlpT_ps[k][:])

            if dbg and layer == 0:
                nc.sync.dma_start(
                    dbg_tensors["dbg_ymlppre"].ap().rearrange(
                        "(k p) t -> p k t", p=P), ymlpT_pre[:])

            # Phase E: 8-way AllReduce of ymlp^T; x = ln(x + ln(ymlp))
            ar2_in = dram.tile([D, T], HDT, name=f"ar2in_{layer}",
                               tag=f"ar2in_{layer}")
            ar2_out = dram.tile([D, T], HDT, name=f"ar2out_{layer}",
                                tag=f"ar2out_{layer}", addr_space="Shared")
            nc.sync.dma_start(
                ar2_in.rearrange("(k p) t -> p k t", p=P), ymlpT_pre[:])
            emit_allreduce(nc, ALL_GROUP, [ar2_in.opt()], [ar2_out.opt()])
            ymlpT_post = arp.tile([P, KD, T], HDT, name="ymlpT_post")
            nc.sync.dma_start(
                ymlpT_post[:], ar2_out.rearrange("(k p) t -> p k t", p=P))
            if dbg and layer == 0:
                nc.sync.dma_start(
                    dbg_tensors["dbg_ymlppost"].ap().rearrange(
                        "(k p) t -> p k t", p=P), ymlpT_post[:])
            with tc.tile_pool(name=f"psE_{layer}", bufs=2,
                              space="PSUM") as psE:
                for j in range(NT):
                    ymt = lnp.tile([P, D], HDT, name="ymt")
                    for k in range(KD):
                        transpose_into(ymt[:, k * P:(k + 1) * P],
                                       ymlpT_post[:, k, j * P:(j + 1) * P],
                                       psE)
                    u = lnp.tile([P, D], F32, name="u_ln")
                    layer_norm(ymt[:], u[:])
                    xn = lnp.tile([P, D], F32, name="xn")
                    nc.vector.tensor_add(xn[:], x_f32[:, j, :], u[:])
                    layer_norm(xn[:], x_f32[:, j, :])
                    set_x_from(j, x_f32, psE)
            if dbg and layer == 0:
                nc.sync.dma_start(
                    dbg_tensors["dbg_x1"].ap().rearrange(
                        "(j p) d -> p j d", p=P), x_f32[:])

        # ---- logits = x @ lm_head ----
        with tc.tile_pool(name="psL", bufs=2, space="PSUM") as psL:
            for j in range(NT):
                ps = psL.tile([P, VOCAB], F32, name="psLt")
                for k in range(KD):
                    nc.tensor.matmul(ps[:],
                                     lhsT=xT_h[:, k, j * P:(j + 1) * P],
                                     rhs=lmh_sb[:, k, :],
                                     start=(k == 0), stop=(k == KD - 1))
                lg = lnp.tile([P, VOCAB], F32, name="lgt")
                nc.scalar.copy(out=lg[:], in_=ps[:])
                nc.sync.dma_start(logits_d.ap()[j * P:(j + 1) * P, :], lg[:])

        for _pool in (statp, lnp, arp, yxp, sdp, schp, ropep, csp,
                      wdec, wenc, dram, persist):
            _pool.release()

    nc.compile()
    return nc


def _host_inputs(idx, embed, encoder, encoder_v, decoder, lm_head):
    """Build the 8 per-core input maps (host-side sharding)."""
    f16 = np.float16
    idx = np.asarray(idx).reshape(-1).astype(np.int64)
    embed = np.asarray(embed, np.float32)
    enc = np.asarray(encoder, np.float32)
    encv = np.asarray(encoder_v, np.float32)
    dec = np.asarray(decoder, np.float32)
    lmh = np.asarray(lm_head, np.float32)

    x0 = embed[idx]  # [T, D] gather on host (pure indexing)

    # freqs exactly as the reference computes them (fp32)
    t = np.arange(0, N, dtype=np.float32)
    q = np.floor(t / 2.0) * 2.0
    freqs = (1.0 / ((2.0 ** 16) ** (q / N)) / TWO_PI).astype(np.float32)
    tvec = np.arange(T, dtype=np.float32)

    umask = (np.arange(P)[:, None] < np.arange(P)[None, :]).astype(np.float32)

    in_maps = []
    for d in range(N_CORES):
        h, half = d // 2, d % 2
        perm = np.concatenate([np.arange(0, NLOC, 2),
                               np.arange(1, NLOC, 2)]) + half * NLOC
        f_loc = freqs[perm[:NLOC // 2]]
        ph = (tvec[None, :] * f_loc[:, None]).astype(np.float32) % 1.0
        in_maps.append({
            "x0": np.ascontiguousarray(x0, np.float32),
            "encw": np.ascontiguousarray(enc[h][:, perm], f16),
            "encvw": np.ascontiguousarray(encv[h][:, perm], f16),
            "decw": np.ascontiguousarray(dec[h * N + perm, :], f16),
            "ct": np.ascontiguousarray(np.cos(TWO_PI * ph), f16),
            "st": np.ascontiguousarray(np.sin(TWO_PI * ph), f16),
            "lmh": np.ascontiguousarray(lmh, f16),
            "umask": umask,
        })
    return in_maps


def kernel(idx, embed, encoder, encoder_v, decoder, lm_head,
           _trace=False, _tmpdir=None):
    if "nc" not in _CACHE:
        _CACHE["nc"] = _build_program()
    nc = _CACHE["nc"]
    in_maps = _host_inputs(idx, embed, encoder, encoder_v, decoder, lm_head)
    res = bass_utils.run_bass_kernel_spmd(
        nc, in_maps, core_ids=list(range(N_CORES)),
        trace=_trace, tmpdir=_tmpdir)
    _CACHE["last_results"] = res
    logits = res.results[0]["logits"].astype(np.float32).reshape(B, T, VOCAB)
    return logits



# revision 40
# speedup vs baseline: 1.0260x; 1.0260x over previous
"""Trainium2 Bass kernel for nn_BDH_1726576853700 (sparse_attention).

3-layer sparse-attention net: B=1, T=1024, D=256, NH=4, N=8192, VOCAB=256.

Sharding over 8 NeuronCores: device d -> (head h=d//2, half=d%2) — each device
owns a 4096-wide slice of one head's sparse latent dim.  Within the slice the
latent index is permuted evens-first so the RoPE pair partner sits exactly 2048
rows away (tile i <-> tile i+16), turning the pair rotation into whole-tile
elementwise ops.  Per layer:
  - x_sparse^T = relu(enc_w^T @ x^T)   (local)
  - qr = rope(x_sparse)                (local, host-precomputed cos/sin tables)
  - S_partial = qr^T qr (local n contraction), strictly-causal masked
  - ykv_partial = S_masked^T @ x ; pair AllReduce (the two halves of one head)
  - ykv_ln = layernorm(ykv); y_sparse^T = relu(encv_w^T @ ykv_ln^T) (local)
  - ymlp^T_partial = dec^T-contracted with (x_sparse * y_sparse)    (local)
  - 8-way AllReduce(ymlp); x = ln(x + ymlp*rstd) (replicated; the inner
    ln(ymlp) mean-shift washes out under the outer LN, so only the 1/std
    scale is applied — numerically identical output)

Schedule notes (what makes this fast):
  - Everything is split along the T dimension into column halves (c=0/1).
    The c=0 score chunks only touch qr columns 0:512, so phase B starts as
    soon as the c=0 half of phase A (+ its RoPE) is done; the c=1 half of
    phase A is interleaved between the c=0 score chunks so the PE never
    waits for the DVE RoPE stream.  Phase D is split the same way so the
    post-AllReduce LN/transpose chains of phase C (j=4..7) overlap the
    c=0 half of phase D, and the phase-E chains (j=4..7) overlap the next
    layer's phase A c=0 half.
  - Phase A walks m in RoPE-pair order (0,16,1,17,...) with the pair's
    rotation emitted inline; score matmuls contract k in the same order.
  - ykv matmuls run one S-chunk behind the score matmuls, and decoder
    matmuls one m behind the encv matmuls (software pipelining keeps the
    in-order PE queue from stalling on the PSUM->SBUF copies between them).
  - ReLUs alternate between the Activation and Pool(GpSimd) engines so the
    activation stream never throttles the PE m-loop.
  - All streamed weights/tables are host-repacked into partition-major tile
    layout so every DMA is a few contiguous >=1KB runs per partition
    (descriptor count, not byte count, dominates DMA cost): encoder tiles
    in pair order, encv+decoder tiles fused per m, cos+sin fused per
    (pair, half).  Tables + AllReduce-result loads go through the
    Activation engine's DGE queue, weights through SP's.
Collectives run in fp16; matmuls in fp16 with fp32 PSUM accumulation; the
residual stream x stays fp32 on-chip.

PSUM discipline: every accumulation group owns its bank(s) exclusively —
`start=True` clears has_written bits for the WHOLE bank.  The decoder
output tiles [P, T] f32 span two banks with the c=0 column half exactly
filling bank 0 and c=1 bank 1, so the per-half accumulation groups stay
bank-exclusive.
"""

import math
import sys

for _p in ("/opt/trn_rl_repo",):
    if _p not in sys.path:
        sys.path.insert(0, _p)

import numpy as np

import concourse.bass as bass
import concourse.mybir as mybir
import concourse.tile as tile
from concourse import bacc, bass_utils
from concourse.masks import make_identity

# ---- problem constants (hardcoded per contract) ----
B, T, D, NH, N = 1, 1024, 256, 4, 8192
VOCAB = 256
N_LAYER = 3
EPS = 1e-5
TWO_PI = 2.0 * math.pi
N_CORES = 8
NLOC = N // 2          # latent columns per device: 4096
P = 128
NT = T // P            # 8 t-tiles
KD = D // P            # 2 d-tiles
NM = NLOC // P         # 32 n-tiles per device
NPAIR = NM // 2        # 16 rope pairs
TH = T // 2            # column half: 512
HDT = mybir.dt.float16     # on-chip activation dtype
F32 = mybir.dt.float32
YKV_SCALE = 1.0 / 256.0    # keeps ykv in fp16 range; LN downstream is
                           # scale-invariant so the result is unchanged
MUL = mybir.AluOpType.mult

# m-tile emission order: rope pair p is complete after tiles p and p+NPAIR
PAIR_ORDER = [m for p in range(NPAIR) for m in (p, p + NPAIR)]
K_ORDER = PAIR_ORDER  # S contraction follows qr production order
ETILE = KD * P             # encoder tile block: 256 cols per m
WTILE = KD * P + D         # fused encv+dec block: 512 cols per m

_CACHE = {}


def _build_program(dbg=False, use_collectives=True, n_layers=N_LAYER):
    def emit_allreduce(nc, groups, ins, outs):
        if use_collectives:
            nc.gpsimd.collective_compute(
                "AllReduce", mybir.AluOpType.add, replica_groups=groups,
                ins=ins, outs=outs)
        else:
            # timing/sim variant: replace the collective with a plain copy
            nc.sync.dma_start(outs[0], ins[0])

    nc = bacc.Bacc("TRN2", target_bir_lowering=False, debug=False,
                   num_devices=N_CORES)
    dbg_tensors = {}
    if dbg:
        for nm, shape, dt in [
            ("dbg_x0ln", [T, D], F32),
            ("dbg_xsp", [NM * P, T], HDT),
            ("dbg_qr", [NM * P, T], HDT),
            ("dbg_ykvpre", [T, D], HDT),
            ("dbg_ykvT", [D, T], HDT),
            ("dbg_x1", [T, D], F32),
        ]:
            dbg_tensors[nm] = nc.dram_tensor(nm, shape, dt,
                                             kind="ExternalOutput")

    x0_d = nc.dram_tensor("x0", [T, D], F32, kind="ExternalInput")
    enc_d = nc.dram_tensor("enc_pk", [P, NM * ETILE], HDT,
                           kind="ExternalInput")
    wv_d = nc.dram_tensor("wv_pk", [P, NM * WTILE], HDT,
                          kind="ExternalInput")
    cst_d = nc.dram_tensor("cst_pk", [P, NPAIR * 2 * T], HDT,
                           kind="ExternalInput")
    lmh_d = nc.dram_tensor("lmh", [D, VOCAB], HDT, kind="ExternalInput")
    umask_d = nc.dram_tensor("umask", [P, P], F32, kind="ExternalInput")
    logits_d = nc.dram_tensor("logits", [T, VOCAB], HDT,
                              kind="ExternalOutput")

    PAIR_GROUPS = [[0, 1], [2, 3], [4, 5], [6, 7]]
    ALL_GROUP = [list(range(N_CORES))]

    with tile.TileContext(nc) as tc:
        persist = tc.alloc_tile_pool(name="persist", bufs=1)
        dram = tc.alloc_tile_pool(name="dram", bufs=1, space="DRAM")

        # persistent SBUF state
        x_sp = persist.tile([P, NM, T], HDT)        # x_sparse^T tiles
        qr = persist.tile([P, NM, T], HDT)          # roped x_sparse^T
        x_f32 = persist.tile([P, NT, D], F32)       # residual stream (natural)
        x_h = persist.tile([P, NT, D], HDT)         # x natural fp16
        xT_h = persist.tile([P, KD, T], HDT)        # x^T fp16
        ykvT_h = persist.tile([P, KD, T], HDT)      # ykv_ln^T fp16
        lmh_sb = persist.tile([P, KD, VOCAB], HDT)
        umask_sb = persist.tile([P, P], F32)
        ident = persist.tile([P, P], HDT)

        eps_sb = persist.tile([P, 1], F32)
        nc.vector.memset(eps_sb[:], float(EPS))
        nc.sync.dma_start(umask_sb[:], umask_d.ap())
        make_identity(nc, ident[:])
        for k in range(KD):
            nc.sync.dma_start(lmh_sb[:, k, :], lmh_d.ap()[k * P:(k + 1) * P, :])

        # streaming / working pools (live across the whole kernel)
        wenc = tc.alloc_tile_pool(name="wenc", bufs=3)
        wdec = tc.alloc_tile_pool(name="wdec", bufs=3)
        csp = tc.alloc_tile_pool(name="csp", bufs=3)
        ropep = tc.alloc_tile_pool(name="ropep", bufs=1)
        schp = tc.alloc_tile_pool(name="schp", bufs=3)
        sdp = tc.alloc_tile_pool(name="sdp", bufs=3)
        yxp = tc.alloc_tile_pool(name="yxp", bufs=3)
        arp = tc.alloc_tile_pool(name="arp", bufs=1)
        cpost = tc.alloc_tile_pool(name="cpost", bufs=2)
        ylp = tc.alloc_tile_pool(name="ylp", bufs=5)
        lnp = tc.alloc_tile_pool(name="lnp", bufs=2)
        statp = tc.alloc_tile_pool(name="statp", bufs=4)

        def ln_rstd(src_ap, statname):
            """mean/var of a [P, D] tile; returns (mv, rstd) where
            rstd = 1/sqrt(var + eps)."""
            stats = statp.tile([P, 6], F32, name=f"{statname}_st")
            mv = statp.tile([P, 2], F32, name=f"{statname}_mv")
            rstd = statp.tile([P, 1], F32, name=f"{statname}_rs")
            nc.vector.bn_stats(out=stats[:], in_=src_ap)
            nc.vector.bn_aggr(out=mv[:], in_=stats[:])
            nc.scalar.activation(out=rstd[:], in_=mv[:, 1:2],
                                 func=mybir.ActivationFunctionType.Sqrt,
                                 bias=eps_sb[:])
            nc.vector.reciprocal(out=rstd[:], in_=rstd[:])
            return mv, rstd

        def layer_norm(src_ap, out_ap, statname="ln"):
            mv, rstd = ln_rstd(src_ap, statname)
            nc.vector.tensor_scalar(out=out_ap, in0=src_ap,
                                    scalar1=mv[:, 0:1], scalar2=rstd[:],
                                    op0=mybir.AluOpType.subtract,
                                    op1=MUL)

        def transpose_into(dst_ap, src_ap, pst_pool, eng):
            """PE-transpose a [P, P] fp16 SBUF block into dst (via PSUM)."""
            pst = pst_pool.tile([P, P], HDT, name="pst")
            nc.tensor.transpose(pst[:], src_ap, ident[:])
            if eng is nc.scalar:
                nc.scalar.copy(out=dst_ap, in_=pst[:])
            else:
                eng.tensor_copy(out=dst_ap, in_=pst[:])

        def set_x_from(j, pst_pool):
            """Write x_h/xT_h for t-tile j from x_f32[:, j, :] (the fp16
            copy runs on the otherwise idle Pool engine — SBUF to SBUF)."""
            nc.gpsimd.tensor_copy(out=x_h[:, j, :], in_=x_f32[:, j, :])
            for k in range(KD):
                transpose_into(xT_h[:, k, j * P:(j + 1) * P],
                               x_h[:, j, k * P:(k + 1) * P], pst_pool,
                               nc.vector if k == 0 else nc.scalar)

        lgt_g = persist.tile([P, NT, VOCAB], HDT)

        def emit_logits(j, psL):
            ps = psL.tile([P, VOCAB], F32, name="psLt")
            for k in range(KD):
                nc.tensor.matmul(ps[:],
                                 lhsT=xT_h[:, k, j * P:(j + 1) * P],
                                 rhs=lmh_sb[:, k, :],
                                 start=(k == 0), stop=(k == KD - 1))
            nc.scalar.copy(out=lgt_g[:, j, :], in_=ps[:])
            if j % 4 == 3:
                nc.sync.dma_start(
                    logits_d.ap().rearrange("(j p) v -> p j v", p=P)[
                        :, j - 3:j + 1, :],
                    lgt_g[:, j - 3:j + 1, :])

        def load_ym_group(ar2_out, j0):
            """One DMA for the AR'd ymlp^T slices of t-tiles j0..j0+3."""
            ymg = cpost.tile([P, KD, 4, P], HDT, name="ym_g")
            nc.scalar.dma_start(
                ymg[:], ar2_out.rearrange("p (k t) -> p k t", k=KD)[
                    :, :, j0 * P:(j0 + 4) * P])
            return ymg

        def chain_E1(ymg, j, psE):
            """Stage 1 of a layer tail for t-tile j: transpose the AR'd
            ymlp^T slice to natural, x = ln(x + ymlp*rstd) into x_f32."""
            ymt = lnp.tile([P, D], F32, name="ymt")
            for k in range(KD):
                transpose_into(ymt[:, k * P:(k + 1) * P], ymg[:, k, j % 4, :],
                               psE, nc.vector if k == 0 else nc.scalar)
            _, rv = ln_rstd(ymt[:], "lne1")
            xpre = lnp.tile([P, D], F32, name="xpre")
            nc.vector.scalar_tensor_tensor(
                out=xpre[:], in0=ymt[:], scalar=rv[:],
                in1=x_f32[:, j, :], op0=MUL, op1=mybir.AluOpType.add)
            layer_norm(xpre[:], x_f32[:, j, :], "lne2")

        def chain_E2(j, psE, psL):
            """Stage 2: refresh x_h/xT_h (+ logits on last layer).  Emitted
            one chain behind stage 1 so the PE never waits on the LN stack."""
            set_x_from(j, psE)
            if psL is not None:
                emit_logits(j, psL)

        def load_yp_group(ar_out, j0):
            """One DMA for the AR'd ykv slices of t-tiles j0..j0+3."""
            ypg = cpost.tile([P, 4, D], HDT, name="ykv_pg")
            nc.scalar.dma_start(
                ypg[:], ar_out.rearrange("p (j d) -> p j d", j=NT)[
                    :, j0:j0 + 4, :])
            return ypg

        def chain_C1(ypg, j):
            """Stage 1 of the ykv tail: LN the AR'd slice.  Returns the
            normalized tile for stage 2."""
            yl = ylp.tile([P, D], HDT, name="ykv_ln")
            layer_norm(ypg[:, j % 4, :], yl[:], "lnc")
            return yl

        def chain_C2(j, yl, psT):
            for k in range(KD):
                transpose_into(ykvT_h[:, k, j * P:(j + 1) * P],
                               yl[:, k * P:(k + 1) * P], psT,
                               nc.vector if k == 0 else nc.scalar)

        def rope_half(p, c, pos):
            """Rotate pair p's columns [c*TH:(c+1)*TH]."""
            cols = slice(c * TH, (c + 1) * TH)
            cstt = csp.tile([P, 2, TH], HDT, name="cstt")
            blk = (p * 2 + c) * 2 * TH
            nc.scalar.dma_start(cstt[:], cst_d.ap()[:, blk:blk + 2 * TH])
            ctt = cstt[:, 0, :]
            stt = cstt[:, 1, :]
            xe = x_sp[:, p, cols]
            xo = x_sp[:, p + NPAIR, cols]
            t1 = ropep.tile([P, TH], HDT, name="rope_t1")
            t2 = ropep.tile([P, TH], HDT, name="rope_t2")
            nc.vector.tensor_mul(t1[:], xe, ctt)
            nc.gpsimd.tensor_mul(t2[:], xo, stt)
            nc.vector.tensor_sub(qr[:, p, cols], t1[:], t2[:])
            t3 = ropep.tile([P, TH], HDT, name="rope_t1")
            t4 = ropep.tile([P, TH], HDT, name="rope_t2")
            nc.vector.tensor_mul(t3[:], xo, ctt)
            nc.vector.tensor_mul(t4[:], xe, stt)
            nc.vector.tensor_add(qr[:, p + NPAIR, cols], t3[:], t4[:])

        def phase_A_positions(c, positions, psA):
            """enc projection + relu + rope for column half c over the given
            PAIR_ORDER positions."""
            cols = slice(c * TH, (c + 1) * TH)
            for pos in positions:
                m = PAIR_ORDER[pos]
                if pos % 2 == 0:
                    et = wenc.tile([P, 2, KD, P], HDT, name="enc_t")
                    nc.sync.dma_start(
                        et[:], enc_d.ap()[:, pos * ETILE:(pos + 2) * ETILE])
                    phase_A_positions.et = et
                et = phase_A_positions.et
                half = pos % 2
                ps = psA.tile([P, TH], F32, name=f"psA{c}")
                for k in range(KD):
                    nc.tensor.matmul(
                        ps[:], lhsT=et[:, half, k, :],
                        rhs=xT_h[:, k, cols],
                        start=(k == 0), stop=(k == KD - 1))
                nc.scalar.activation(
                    out=x_sp[:, m, cols], in_=ps[:],
                    func=mybir.ActivationFunctionType.Relu)
                if pos % 2 == 1:
                    rope_half(pos // 2, c, pos)

        def s_chunk(c, i, psS):
            """Score chunk (c, i): 32 causal matmuls + f16 copy + diag mask.
            Returns (i, sc, sd, base) for the lag-1 ykv emission."""
            base = max(c * TH, i * P)
            width = (c + 1) * TH - base
            ps = psS.tile([P, TH], F32, name=f"psS{c}")
            nk = len(K_ORDER)
            for ki, k in enumerate(K_ORDER):
                nc.tensor.matmul(
                    ps[:, :width],
                    lhsT=qr[:, k, i * P:(i + 1) * P],
                    rhs=qr[:, k, base:base + width],
                    start=(ki == 0), stop=(ki == nk - 1))
            sc = schp.tile([P, TH], HDT, name="schunk")
            if i % 2 == 0:
                nc.scalar.copy(out=sc[:, :width], in_=ps[:, :width])
            else:
                nc.vector.tensor_copy(out=sc[:, :width], in_=ps[:, :width])
            sd = None
            if c == i // 4:
                dcol = i * P - base
                sd = sdp.tile([P, P], HDT, name="sdiag")
                nc.vector.tensor_mul(sd[:], ps[:, dcol:dcol + P],
                                     umask_sb[:])
            return (i, sc, sd, base)

        # ---- initial x = ln(embed[idx]) (gather done on host into x0) ----
        with tc.tile_pool(name="ps_init", bufs=2, space="PSUM") as ps_init:
            for j in range(NT):
                if j % 4 == 0:
                    x0g = cpost.tile([P, 4, D], F32, name="x0g", bufs=1)
                    nc.sync.dma_start(
                        x0g[:], x0_d.ap().rearrange(
                            "(j p) d -> p j d", p=P)[:, j:j + 4, :])
                layer_norm(x0g[:, j % 4, :], x_f32[:, j, :], "ln0")
                if j > 0:
                    set_x_from(j - 1, ps_init)
            set_x_from(NT - 1, ps_init)
        if dbg:
            nc.sync.dma_start(
                dbg_tensors["dbg_x0ln"].ap().rearrange("(j p) d -> p j d", p=P),
                x_f32[:])

        # ---- layers ----
        prev_tail = None
        for layer in range(n_layers):
            # Phase A c=0 (+ previous layer's E chains j=4..7 interleaved at
            # pair-group boundaries, stage 2 one group behind stage 1 so the
            # set_x transposes never make the PE wait on the LN stacks).
            with tc.tile_pool(name=f"psA0_{layer}", bufs=4,
                              space="PSUM") as psA0, \
                 tc.tile_pool(name=f"psE2_{layer}", bufs=2,
                              space="PSUM") as psE2:
                tail_ymg = None
                for g in range(4):
                    phase_A_positions(0, range(8 * g, 8 * g + 8), psA0)
                    if prev_tail is not None:
                        if g == 0:
                            tail_ymg = load_ym_group(prev_tail, 4)
                        else:
                            chain_E2(3 + g, psE2, None)
                        chain_E1(tail_ymg, 4 + g, psE2)
                if prev_tail is not None:
                    chain_E2(7, psE2, None)
            prev_tail = None

            # Phase B c=0 interleaved with phase A c=1: the c=0 score chunks
            # only read qr columns 0:512.  Between chunks, a quarter of the
            # phase-A c=1 m-loop runs so its relu/rope stream stays ahead of
            # the c=1 score chunks that follow.  Each ykv accumulator is
            # scaled and DMA'd out to the AllReduce buffer the moment its
            # last matmul stops, so the collective fires right after the
            # final chunk instead of after a bulk copy.
            ykv_pre = arp.tile([P, NT, D], HDT, name="ykv_pre")
            ar_in = dram.tile([P, NT * D], HDT, name=f"arin_{layer}",
                              tag=f"arin_{layer}")

            def emit_ykv(ykv_ps, c, i, sc, sd, base):
                for j in range(max(4 * c, i), 4 * c + 4):
                    lhsT = sd[:] if j == i else \
                        sc[:, j * P - base:(j + 1) * P - base]
                    nc.tensor.matmul(
                        ykv_ps[j - 4 * c][:], lhsT=lhsT,
                        rhs=x_h[:, i, :],
                        start=(i == 0), stop=(i == j))
                if i >= 4 * c:
                    nc.scalar.mul(out=ykv_pre[:, i, :],
                                  in_=ykv_ps[i - 4 * c][:], mul=YKV_SCALE)
                    nc.sync.dma_start(ar_in[:, i * D:(i + 1) * D],
                                      ykv_pre[:, i, :])

            with tc.tile_pool(name=f"psS0_{layer}", bufs=2,
                              space="PSUM") as psS0, \
                 tc.tile_pool(name=f"psY0_{layer}", bufs=1,
                              space="PSUM") as psY0, \
                 tc.tile_pool(name=f"psA1_{layer}", bufs=2,
                              space="PSUM") as psA1:
                ykv_ps = [psY0.tile([P, D], F32, name=f"ykv_ps{j}",
                                    tag=f"ykv_ps{j}") for j in range(4)]
                pend = None
                for g in range(4):
                    args = s_chunk(0, g, psS0)
                    if pend is not None:
                        emit_ykv(ykv_ps, 0, *pend)
                    pend = args
                    phase_A_positions(1, range(8 * g, 8 * g + 8), psA1)
                emit_ykv(ykv_ps, 0, *pend)

            # Phase B c=1: all eight score chunks, ykv one chunk behind.
            with tc.tile_pool(name=f"psS1_{layer}", bufs=4,
                              space="PSUM") as psS1, \
                 tc.tile_pool(name=f"psY1_{layer}", bufs=1,
                              space="PSUM") as psY1:
                ykv_ps = [psY1.tile([P, D], F32, name=f"ykv_ps{j}",
                                    tag=f"ykv_ps{j}") for j in range(4, 8)]
                pend = None
                for i in range(8):
                    args = s_chunk(1, i, psS1)
                    if pend is not None:
                        emit_ykv(ykv_ps, 1, *pend)
                    pend = args
                emit_ykv(ykv_ps, 1, *pend)

            if dbg and layer == 0:
                nc.sync.dma_start(
                    dbg_tensors["dbg_xsp"].ap().rearrange(
                        "(m p) t -> p m t", p=P), x_sp[:])
                nc.sync.dma_start(
                    dbg_tensors["dbg_qr"].ap().rearrange(
                        "(m p) t -> p m t", p=P), qr[:])
                nc.sync.dma_start(
                    dbg_tensors["dbg_ykvpre"].ap().rearrange(
                        "(j p) d -> p j d", p=P), ykv_pre[:])

            # Phase C: pair AllReduce of ykv; per-t-tile LN+transpose chains
            # two-staged (stage 1 has no PE work, so all loads+LNs fire
            # first and the transposes stream without per-chain stalls);
            # j=4..7 overlapped with phase D's c=0 half.
            ar_out = dram.tile([P, NT * D], HDT, name=f"arout_{layer}",
                               tag=f"arout_{layer}")
            emit_allreduce(nc, PAIR_GROUPS, [ar_in.opt()], [ar_out.opt()])

            ymlpT_pre = arp.tile([P, KD, T], HDT, name="ymlpT_pre")
            ar2_in = dram.tile([P, KD * T], HDT, name=f"ar2in_{layer}",
                               tag=f"ar2in_{layer}")
            with tc.tile_pool(name=f"psT_{layer}", bufs=2,
                              space="PSUM") as psT:
                ypg = load_yp_group(ar_out, 0)
                yls = [chain_C1(ypg, j) for j in range(4)]
                for j in range(4):
                    chain_C2(j, yls[j], psT)
                with tc.tile_pool(name=f"psD_{layer}", bufs=2,
                                  space="PSUM") as psD, \
                     tc.tile_pool(name=f"psM_{layer}", bufs=1,
                                  space="PSUM") as psM:
                    ymlpT_ps = [psM.tile([P, T], F32, name=f"ymlpT_ps{k}",
                                         tag=f"ymlpT_ps{k}")
                                for k in range(KD)]

                    def emit_dec(cols, m, wvt, half, xy):
                        for k in range(KD):
                            nc.tensor.matmul(
                                ymlpT_ps[k][:, cols],
                                lhsT=wvt[:, half,
                                         ETILE + k * P:ETILE + (k + 1) * P],
                                rhs=xy[:],
                                start=(m == 0), stop=(m == NM - 1))

                    for c in range(2):
                        cols = slice(c * TH, (c + 1) * TH)
                        pend = None
                        wvt = None
                        for m in range(NM):
                            if m % 2 == 0:
                                wvt = wdec.tile([P, 2, WTILE], HDT,
                                                name="wv_t")
                                nc.sync.dma_start(
                                    wvt[:],
                                    wv_d.ap()[:, m * WTILE:(m + 2) * WTILE])
                            half = m % 2
                            ps = psD.tile([P, TH], F32, name="psD")
                            for k in range(KD):
                                nc.tensor.matmul(
                                    ps[:],
                                    lhsT=wvt[:, half, k * P:(k + 1) * P],
                                    rhs=ykvT_h[:, k, cols],
                                    start=(k == 0), stop=(k == KD - 1))
                            ysp = yxp.tile([P, TH], HDT, name="ysp",
                                           bufs=2)
                            nc.scalar.activation(
                                out=ysp[:], in_=ps[:],
                                func=mybir.ActivationFunctionType.Relu)
                            xy = yxp.tile([P, TH], HDT, name="xy")
                            nc.vector.tensor_mul(xy[:], ysp[:],
                                                 x_sp[:, m, cols])
                            if pend is not None:
                                emit_dec(cols, *pend)
                            pend = (m, wvt, half, xy)
                        emit_dec(cols, *pend)
                        if c == 0:
                            ypg = load_yp_group(ar_out, 4)
                            yls = [chain_C1(ypg, j) for j in range(4, NT)]
                            for j in range(4, NT):
                                chain_C2(j, yls[j - 4], psT)
                    for k in range(KD):
                        if k == 0:
                            nc.scalar.copy(out=ymlpT_pre[:, k, :],
                                           in_=ymlpT_ps[k][:])
                        else:
                            nc.vector.tensor_copy(out=ymlpT_pre[:, k, :],
                                                  in_=ymlpT_ps[k][:])
                        nc.sync.dma_start(ar2_in[:, k * T:(k + 1) * T],
                                          ymlpT_pre[:, k, :])

            if dbg and layer == 0:
                nc.sync.dma_start(
                    dbg_tensors["dbg_ykvT"].ap().rearrange(
                        "(k p) t -> p k t", p=P), ykvT_h[:])

            # Phase E: 8-way AllReduce of ymlp^T; x = ln(x + ymlp*rstd).
            # Chains j=0..3 here (stage 2 one chain behind stage 1);
            # j=4..7 ride inside the next layer's phase A c=0 (prev_tail).
            # Last layer emits everything plus logits inline.
            ar2_out = dram.tile([P, KD * T], HDT, name=f"ar2out_{layer}",
                                tag=f"ar2out_{layer}", addr_space="Shared")
            emit_allreduce(nc, ALL_GROUP, [ar2_in.opt()], [ar2_out.opt()])

            last = layer == n_layers - 1
            if last:
                psL = tc.alloc_tile_pool(name="psL", bufs=2, space="PSUM")
                with tc.tile_pool(name=f"psE_{layer}", bufs=2,
                                  space="PSUM") as psE:
                    for j in range(NT):
                        if j % 4 == 0:
                            ymg = load_ym_group(ar2_out, j)
                        chain_E1(ymg, j, psE)
                        if j > 0:
                            chain_E2(j - 1, psE, psL)
                    chain_E2(NT - 1, psE, psL)
                psL.release()
            else:
                with tc.tile_pool(name=f"psE_{layer}", bufs=2,
                                  space="PSUM") as psE:
                    ymg = load_ym_group(ar2_out, 0)
                    for j in range(4):
                        chain_E1(ymg, j, psE)
                        if j > 0:
                            chain_E2(j - 1, psE, None)
                    chain_E2(3, psE, None)

                prev_tail = ar2_out

            if dbg and layer == 0:
                nc.sync.dma_start(
                    dbg_tensors["dbg_x1"].ap().rearrange(
                        "(j p) d -> p j d", p=P), x_f32[:])

        for _pool in (statp, lnp, ylp, cpost, arp, yxp, sdp, schp, ropep,
                      csp, wdec, wenc, dram, persist):
            _pool.release()

    nc.compile()
    return nc


def _host_inputs(idx, embed, encoder, encoder_v, decoder, lm_head):
    """Build the 8 per-core input maps (host-side sharding + tile packing)."""
    f16 = np.float16
    idx = np.asarray(idx).reshape(-1).astype(np.int64)
    embed = np.asarray(embed, np.float32)
    enc = np.asarray(encoder, np.float32)
    encv = np.asarray(encoder_v, np.float32)
    dec = np.asarray(decoder, np.float32)
    lmh = np.asarray(lm_head, np.float32)

    x0 = embed[idx]  # [T, D] gather on host (pure indexing)

    # freqs exactly as the reference computes them (fp32)
    t = np.arange(0, N, dtype=np.float32)
    q = np.floor(t / 2.0) * 2.0
    freqs = (1.0 / ((2.0 ** 16) ** (q / N)) / TWO_PI).astype(np.float32)
    tvec = np.arange(T, dtype=np.float32)

    umask = (np.arange(P)[:, None] < np.arange(P)[None, :]).astype(np.float32)

    def tileize(w):
        """[D, NLOC] weight -> [P, NM, KD, P] partition-major tiles:
        out[p, m, k, n] = w[k*P + p, m*P + n]."""
        return np.ascontiguousarray(
            w.reshape(KD, P, NM, P).transpose(1, 2, 0, 3))

    in_maps = []
    for d in range(N_CORES):
        h, half = d // 2, d % 2
        perm = np.concatenate([np.arange(0, NLOC, 2),
                               np.arange(1, NLOC, 2)]) + half * NLOC
        f_loc = freqs[perm[:NLOC // 2]]
        ph = (tvec[None, :] * f_loc[:, None]).astype(np.float32) % 1.0
        ct = np.cos(TWO_PI * ph).astype(f16)   # [NPAIR*P, T]
        st = np.sin(TWO_PI * ph).astype(f16)
        # cos/sin fused per (pair, column-half): [P, NPAIR, 2, 2, TH]
        cst = np.stack([ct.reshape(NPAIR, P, 2, TH),
                        st.reshape(NPAIR, P, 2, TH)],
                       axis=3).transpose(1, 0, 2, 3, 4)

        enc_t = tileize(enc[h][:, perm].astype(f16))[:, PAIR_ORDER]
        encv_t = tileize(encv[h][:, perm].astype(f16))  # [P, NM, KD, P]
        # decoder tiles: [P, NM, D]; row p of tile m is latent m*P+p
        dec_t = dec[h * N + perm, :].astype(f16).reshape(NM, P, D)
        dec_t = np.ascontiguousarray(dec_t.transpose(1, 0, 2))
        # fused encv+dec per m: [P, NM, WTILE]
        wv = np.concatenate([encv_t.reshape(P, NM, ETILE), dec_t], axis=2)

        in_maps.append({
            "x0": np.ascontiguousarray(x0, np.float32),
            "enc_pk": np.ascontiguousarray(enc_t.reshape(P, NM * ETILE)),
            "wv_pk": np.ascontiguousarray(wv.reshape(P, NM * WTILE)),
            "cst_pk": np.ascontiguousarray(cst.reshape(P, NPAIR * 2 * T)),
            "lmh": np.ascontiguousarray(lmh, f16),
            "umask": umask,
        })
    return in_maps


def kernel(idx, embed, encoder, encoder_v, decoder, lm_head,
           _trace=False, _tmpdir=None):
    if "nc" not in _CACHE:
        _CACHE["nc"] = _build_program()
    nc = _CACHE["nc"]
    in_maps = _host_inputs(idx, embed, encoder, encoder_v, decoder, lm_head)
    res = bass_utils.run_bass_kernel_spmd(
        nc, in_maps, core_ids=list(range(N_CORES)),
        trace=_trace, tmpdir=_tmpdir)
    _CACHE["last_results"] = res
    logits = res.results[0]["logits"].astype(np.float32).reshape(B, T, VOCAB)
    return logits


# revision 68
# speedup vs baseline: 1.1214x; 1.0931x over previous
"""Trainium2 Bass kernel for nn_BDH_1726576853700 (sparse_attention).

3-layer sparse-attention net: B=1, T=1024, D=256, NH=4, N=8192, VOCAB=256.

Sharding over 8 NeuronCores: device d -> (head h=d//2, half=d%2) — each device
owns a 4096-wide slice of one head's sparse latent dim.  Within the slice the
latent index is permuted evens-first so the RoPE pair partner sits exactly 2048
rows away (tile i <-> tile i+16), turning the pair rotation into whole-tile
elementwise ops.  Per layer:
  - x_sparse^T = relu(enc_w^T @ x^T)   (local)
  - qr = rope(x_sparse)                (local, host-precomputed cos/sin tables)
  - S_partial = qr^T qr (local n contraction), strictly-causal masked
  - ykv_partial = S_masked^T @ x ; pair AllReduce (the two halves of one head)
  - ykv_ln = layernorm(ykv); y_sparse^T = relu(encv_w^T @ ykv_ln^T) (local)
  - ymlp^T_partial = dec^T-contracted with (x_sparse * y_sparse)    (local)
  - 8-way AllReduce(ymlp); x = ln(x + ymlp*rstd) (replicated; the inner
    ln(ymlp) mean-shift washes out under the outer LN, so only the 1/std
    scale is applied — numerically identical output)

Schedule notes (what makes this fast):
  - Everything is split along the T dimension into column halves (c=0/1).
    The c=0 score chunks only touch qr columns 0:512, so phase B starts as
    soon as the c=0 half of phase A (+ its RoPE) is done; the c=1 half of
    phase A is interleaved between the c=0 score chunks so the PE never
    waits for the DVE RoPE stream.  Phase D is split the same way so the
    post-AllReduce LN/transpose chains of phase C (j=4..7) overlap the
    c=0 half of phase D, and the phase-E chains (j=4..7) overlap the next
    layer's phase A c=0 half.
  - Phase A walks m in RoPE-pair order (0,16,1,17,...) with the pair's
    rotation emitted inline; score matmuls contract k in the same order.
  - ykv matmuls run one S-chunk behind the score matmuls, and decoder
    matmuls one m behind the encv matmuls (software pipelining keeps the
    in-order PE queue from stalling on the PSUM->SBUF copies between them).
  - ReLUs alternate between the Activation and Pool(GpSimd) engines so the
    activation stream never throttles the PE m-loop.
  - All streamed weights/tables are host-repacked into partition-major tile
    layout so every DMA is a few contiguous >=1KB runs per partition
    (descriptor count, not byte count, dominates DMA cost): encoder tiles
    in pair order, encv+decoder tiles fused per m, cos+sin fused per
    (pair, half).  Tables + AllReduce-result loads go through the
    Activation engine's DGE queue, weights through SP's.
Collectives run in fp16; matmuls in fp16 with fp32 PSUM accumulation; the
residual stream x stays fp32 on-chip.

PSUM discipline: every accumulation group owns its bank(s) exclusively —
`start=True` clears has_written bits for the WHOLE bank.  The decoder
output tiles [P, T] f32 span two banks with the c=0 column half exactly
filling bank 0 and c=1 bank 1, so the per-half accumulation groups stay
bank-exclusive.
"""

import math
import sys

for _p in ("/opt/trn_rl_repo",):
    if _p not in sys.path:
        sys.path.insert(0, _p)

import numpy as np

import concourse.bass as bass
import concourse.mybir as mybir
import concourse.tile as tile
from concourse import bacc, bass_utils
from concourse.masks import make_identity

# ---- problem constants (hardcoded per contract) ----
B, T, D, NH, N = 1, 1024, 256, 4, 8192
VOCAB = 256
N_LAYER = 3
EPS = 1e-5
TWO_PI = 2.0 * math.pi
N_CORES = 8
NLOC = N // 2          # latent columns per device: 4096
P = 128
NT = T // P            # 8 t-tiles
KD = D // P            # 2 d-tiles
NM = NLOC // P         # 32 n-tiles per device
NPAIR = NM // 2        # 16 rope pairs
TH = T // 2            # column half: 512
HDT = mybir.dt.float16     # on-chip activation dtype
F32 = mybir.dt.float32
YKV_SCALE = 1.0 / 256.0    # keeps ykv in fp16 range; LN downstream is
                           # scale-invariant so the result is unchanged
MUL = mybir.AluOpType.mult

# m-tile emission order: rope pair p is complete after tiles p and p+NPAIR
PAIR_ORDER = [m for p in range(NPAIR) for m in (p, p + NPAIR)]
K_ORDER = PAIR_ORDER  # S contraction follows qr production order
ETILE = KD * P             # encoder tile block: 256 cols per m
WTILE = KD * P + D         # fused encv+dec block: 512 cols per m

_CACHE = {}


def _build_program(dbg=False, use_collectives=True, n_layers=N_LAYER):
    def emit_allreduce(nc, groups, ins, outs):
        if use_collectives:
            nc.gpsimd.collective_compute(
                "AllReduce", mybir.AluOpType.add, replica_groups=groups,
                ins=ins, outs=outs)
        else:
            # timing/sim variant: replace the collective with a plain copy
            nc.sync.dma_start(outs[0], ins[0])

    nc = bacc.Bacc("TRN2", target_bir_lowering=False, debug=False,
                   num_devices=N_CORES)
    dbg_tensors = {}
    if dbg:
        for nm, shape, dt in [
            ("dbg_x0ln", [T, D], F32),
            ("dbg_xsp", [NM * P, T], HDT),
            ("dbg_qr", [NM * P, T], HDT),
            ("dbg_ykvpre", [T, D], HDT),
            ("dbg_ykvT", [D, T], HDT),
            ("dbg_x1", [T, D], F32),
        ]:
            dbg_tensors[nm] = nc.dram_tensor(nm, shape, dt,
                                             kind="ExternalOutput")

    x0_d = nc.dram_tensor("x0", [T, D], HDT, kind="ExternalInput")
    enc_d = nc.dram_tensor("enc_pk", [P, NM * ETILE], HDT,
                           kind="ExternalInput")
    wv_d = nc.dram_tensor("wv_pk", [P, NM * WTILE], HDT,
                          kind="ExternalInput")
    cst_d = nc.dram_tensor("cst_pk", [P, NPAIR * 2 * T], HDT,
                           kind="ExternalInput")
    lmh_d = nc.dram_tensor("lmh", [D, VOCAB], HDT, kind="ExternalInput")
    umask_d = nc.dram_tensor("umask", [P, P], F32, kind="ExternalInput")
    logits_d = nc.dram_tensor("logits", [T, VOCAB], HDT,
                              kind="ExternalOutput")

    PAIR_GROUPS = [[0, 1], [2, 3], [4, 5], [6, 7]]
    ALL_GROUP = [list(range(N_CORES))]

    with tile.TileContext(nc) as tc:
        persist = tc.alloc_tile_pool(name="persist", bufs=1)
        dram = tc.alloc_tile_pool(name="dram", bufs=1, space="DRAM")

        # persistent SBUF state
        x_sp = persist.tile([P, NM, T], HDT)        # x_sparse^T tiles
        qr = persist.tile([P, NM, T], HDT)          # roped x_sparse^T
        x_f32 = persist.tile([P, NT, D], F32)       # residual stream (natural)
        x_h = persist.tile([P, NT, D], HDT)         # x natural fp16
        xT_h = persist.tile([P, KD, T], HDT)        # x^T fp16
        ykvT_h = persist.tile([P, KD, T], HDT)      # ykv_ln^T fp16
        lmh_sb = persist.tile([P, KD, VOCAB], HDT)
        umask_sb = persist.tile([P, P], F32)
        ident = persist.tile([P, P], HDT)

        eps_sb = persist.tile([P, 1], F32)
        nc.vector.memset(eps_sb[:], float(EPS))
        nc.sync.dma_start(umask_sb[:], umask_d.ap())
        make_identity(nc, ident[:])
        for k in range(KD):
            nc.sync.dma_start(lmh_sb[:, k, :], lmh_d.ap()[k * P:(k + 1) * P, :])

        # streaming / working pools (live across the whole kernel)
        wenc = tc.alloc_tile_pool(name="wenc", bufs=3)
        wdec = tc.alloc_tile_pool(name="wdec", bufs=3)
        csp = tc.alloc_tile_pool(name="csp", bufs=4)
        ropep = tc.alloc_tile_pool(name="ropep", bufs=1)
        schp = tc.alloc_tile_pool(name="schp", bufs=3)
        sdp = tc.alloc_tile_pool(name="sdp", bufs=2)
        yxp = tc.alloc_tile_pool(name="yxp", bufs=3)
        arp = tc.alloc_tile_pool(name="arp", bufs=1)
        cpost = tc.alloc_tile_pool(name="cpost", bufs=2)
        ylp = tc.alloc_tile_pool(name="ylp", bufs=4)
        lnp = tc.alloc_tile_pool(name="lnp", bufs=2)
        statp = tc.alloc_tile_pool(name="statp", bufs=4)

        def ln_rstd(src_ap, statname):
            """mean/var of a [P, D] tile; returns (mv, rstd) where
            rstd = 1/sqrt(var + eps)."""
            stats = statp.tile([P, 6], F32, name=f"{statname}_st")
            mv = statp.tile([P, 2], F32, name=f"{statname}_mv")
            rstd = statp.tile([P, 1], F32, name=f"{statname}_rs")
            nc.vector.bn_stats(out=stats[:], in_=src_ap)
            nc.vector.bn_aggr(out=mv[:], in_=stats[:])
            nc.scalar.activation(out=rstd[:], in_=mv[:, 1:2],
                                 func=mybir.ActivationFunctionType.Sqrt,
                                 bias=eps_sb[:])
            nc.vector.reciprocal(out=rstd[:], in_=rstd[:])
            return mv, rstd

        def layer_norm(src_ap, out_ap, statname="ln"):
            mv, rstd = ln_rstd(src_ap, statname)
            nc.vector.tensor_scalar(out=out_ap, in0=src_ap,
                                    scalar1=mv[:, 0:1], scalar2=rstd[:],
                                    op0=mybir.AluOpType.subtract,
                                    op1=MUL)

        def transpose_into(dst_ap, src_ap, pst_pool, eng):
            """PE-transpose a [P, P] fp16 SBUF block into dst (via PSUM)."""
            pst = pst_pool.tile([P, P], HDT, name="pst")
            nc.tensor.transpose(pst[:], src_ap, ident[:])
            if eng is nc.scalar:
                nc.scalar.copy(out=dst_ap, in_=pst[:])
            else:
                eng.tensor_copy(out=dst_ap, in_=pst[:])

        def set_x_from(j, pst_pool):
            """Write x_h/xT_h for t-tile j from x_f32[:, j, :] (the fp16
            copy runs on the otherwise idle Pool engine — SBUF to SBUF)."""
            nc.gpsimd.tensor_copy(out=x_h[:, j, :], in_=x_f32[:, j, :])
            for k in range(KD):
                transpose_into(xT_h[:, k, j * P:(j + 1) * P],
                               x_h[:, j, k * P:(k + 1) * P], pst_pool,
                               nc.vector if k == 0 else nc.scalar)

        def emit_logits(j, psL):
            ps = psL.tile([P, VOCAB], F32, name="psLt")
            for k in range(KD):
                nc.tensor.matmul(ps[:],
                                 lhsT=xT_h[:, k, j * P:(j + 1) * P],
                                 rhs=lmh_sb[:, k, :],
                                 start=(k == 0), stop=(k == KD - 1))
            if j % 4 == 0:
                emit_logits.lg = cpost.tile([P, 4, VOCAB], HDT,
                                            name="lgt_g", bufs=1)
            nc.scalar.copy(out=emit_logits.lg[:, j % 4, :], in_=ps[:])
            if j % 4 == 3:
                nc.sync.dma_start(
                    logits_d.ap().rearrange("(j p) v -> p j v", p=P)[
                        :, j - 3:j + 1, :],
                    emit_logits.lg[:])

        def load_ym_group(ar2_out, j0):
            """One DMA for the AR'd ymlp^T slices of t-tiles j0..j0+3."""
            ymg = cpost.tile([P, KD, 4, P], HDT, name="ym_g")
            nc.scalar.dma_start(
                ymg[:], ar2_out.rearrange("p (k t) -> p k t", k=KD)[
                    :, :, j0 * P:(j0 + 4) * P])
            return ymg

        def chain_E1(ymg, j, psE):
            """Stage 1 of a layer tail for t-tile j: transpose the AR'd
            ymlp^T slice to natural, x = ln(x + ymlp*rstd) into x_f32."""
            ymt = lnp.tile([P, D], F32, name="ymt")
            for k in range(KD):
                transpose_into(ymt[:, k * P:(k + 1) * P], ymg[:, k, j % 4, :],
                               psE, nc.vector if k == 0 else nc.scalar)
            _, rv = ln_rstd(ymt[:], "lne1")
            xpre = lnp.tile([P, D], F32, name="xpre")
            nc.vector.scalar_tensor_tensor(
                out=xpre[:], in0=ymt[:], scalar=rv[:],
                in1=x_f32[:, j, :], op0=MUL, op1=mybir.AluOpType.add)
            layer_norm(xpre[:], x_f32[:, j, :], "lne2")

        def chain_E2(j, psE, psL):
            """Stage 2: refresh x_h/xT_h (+ logits on last layer).  Emitted
            one chain behind stage 1 so the PE never waits on the LN stack."""
            set_x_from(j, psE)
            if psL is not None:
                emit_logits(j, psL)

        def load_yp_group(ar_out, j0):
            """One DMA for the AR'd ykv slices of t-tiles j0..j0+3."""
            ypg = cpost.tile([P, 4, D], HDT, name="ykv_pg")
            nc.scalar.dma_start(
                ypg[:], ar_out.rearrange("p (j d) -> p j d", j=NT)[
                    :, j0:j0 + 4, :])
            return ypg

        def chain_C1(ypg, j):
            """Stage 1 of the ykv tail: LN the AR'd slice.  Returns the
            normalized tile for stage 2."""
            yl = ylp.tile([P, D], HDT, name="ykv_ln")
            layer_norm(ypg[:, j % 4, :], yl[:], "lnc")
            return yl

        def chain_C2(j, yl, psT):
            for k in range(KD):
                transpose_into(ykvT_h[:, k, j * P:(j + 1) * P],
                               yl[:, k * P:(k + 1) * P], psT,
                               nc.vector if k == 0 else nc.scalar)

        def rope_half(p, c, cstt):
            """Rotate pair p's columns [c*TH:(c+1)*TH]."""
            cols = slice(c * TH, (c + 1) * TH)
            ctt = cstt[:, 0, :]
            stt = cstt[:, 1, :]
            xe = x_sp[:, p, cols]
            xo = x_sp[:, p + NPAIR, cols]
            t1 = ropep.tile([P, TH], HDT, name="rope_t1")
            t2 = ropep.tile([P, TH], HDT, name="rope_t2")
            nc.vector.tensor_mul(t1[:], xe, ctt)
            nc.gpsimd.tensor_mul(t2[:], xo, stt)
            nc.vector.tensor_sub(qr[:, p, cols], t1[:], t2[:])
            t3 = ropep.tile([P, TH], HDT, name="rope_t1")
            t4 = ropep.tile([P, TH], HDT, name="rope_t2")
            nc.vector.tensor_mul(t3[:], xo, ctt)
            nc.vector.tensor_mul(t4[:], xe, stt)
            nc.vector.tensor_add(qr[:, p + NPAIR, cols], t3[:], t4[:])

        class PhaseA:
            """enc projection + relu + rope for one column half.

            DMA-starts share an in-order queue with their WAR waits, so a
            table load that waits on the rope stream would block the weight
            loads queued behind it.  Each DMA is therefore emitted only
            once its WAR is already resolved: weights three groups ahead of
            the matmuls, cos/sin tables four pairs ahead of the rope."""

            def __init__(self, c, psA):
                self.c = c
                self.psA = psA
                self.ets = {}
                self.cstts = {}

            def _load_et(self, g):
                if g * 4 >= NM:
                    return
                et = wenc.tile([P, 4, KD, P], HDT, name="enc_t")
                nc.sync.dma_start(
                    et[:],
                    enc_d.ap()[:, g * 4 * ETILE:(g + 1) * 4 * ETILE])
                self.ets[g] = et

            def _load_cst(self, p):
                if p >= NPAIR:
                    return
                cstt = csp.tile([P, 2, TH], HDT, name="cstt")
                blk = (p * 2 + self.c) * 2 * TH
                nc.sync.dma_start(cstt[:], cst_d.ap()[:, blk:blk + 2 * TH])
                self.cstts[p] = cstt

            def positions(self, rng):
                cols = slice(self.c * TH, (self.c + 1) * TH)
                for pos in rng:
                    m = PAIR_ORDER[pos]
                    if pos % 4 == 0:
                        self._load_et(pos // 4)
                    et = self.ets[pos // 4]
                    ps = self.psA.tile([P, TH], F32, name=f"psA{self.c}")
                    for k in range(KD):
                        nc.tensor.matmul(
                            ps[:], lhsT=et[:, pos % 4, k, :],
                            rhs=xT_h[:, k, cols],
                            start=(k == 0), stop=(k == KD - 1))
                    nc.scalar.activation(
                        out=x_sp[:, m, cols], in_=ps[:],
                        func=mybir.ActivationFunctionType.Relu)
                    if pos % 2 == 1:
                        p = pos // 2
                        self._load_cst(p)
                        rope_half(p, self.c, self.cstts.pop(p))

        NKA = 24  # contraction split: pass a = first 24 k-tiles (12 pairs)

        def s_chunk_a(c, i, psS, nk=NKA):
            """Score chunk (c, i), pass a: the first nk contraction tiles
            (accumulation left open)."""
            base = max(c * TH, i * P)
            width = (c + 1) * TH - base
            ps = psS.tile([P, TH], F32, name=f"psS{c}")
            for ki in range(nk):
                nc.tensor.matmul(
                    ps[:, :width],
                    lhsT=qr[:, K_ORDER[ki], i * P:(i + 1) * P],
                    rhs=qr[:, K_ORDER[ki], base:base + width],
                    start=(ki == 0), stop=False)
            return ps

        def s_chunk_b(c, i, ps, nk0=NKA):
            """Pass b: remaining contraction tiles + f16 copy + diag mask.
            Returns (i, sc, sd, base) for the lagged ykv emission."""
            base = max(c * TH, i * P)
            width = (c + 1) * TH - base
            nk = len(K_ORDER)
            for ki in range(nk0, nk):
                nc.tensor.matmul(
                    ps[:, :width],
                    lhsT=qr[:, K_ORDER[ki], i * P:(i + 1) * P],
                    rhs=qr[:, K_ORDER[ki], base:base + width],
                    start=False, stop=(ki == nk - 1))
            sc = schp.tile([P, TH], HDT, name="schunk")
            if i % 2 == 0:
                nc.scalar.copy(out=sc[:, :width], in_=ps[:, :width])
            else:
                nc.vector.tensor_copy(out=sc[:, :width], in_=ps[:, :width])
            sd = None
            if c == i // 4:
                dcol = i * P - base
                sd = sdp.tile([P, P], HDT, name="sdiag")
                nc.vector.tensor_mul(sd[:], ps[:, dcol:dcol + P],
                                     umask_sb[:])
            return (i, sc, sd, base)

        def s_chunk(c, i, psS):
            """Single-pass score chunk (for when qr is fully available)."""
            ps = s_chunk_a(c, i, psS, nk=NKA)
            return s_chunk_b(c, i, ps, nk0=NKA)

        # ---- initial x = ln(embed[idx]) (gather done on host into x0) ----
        with tc.tile_pool(name="ps_init", bufs=2, space="PSUM") as ps_init:
            for j in range(NT):
                if j % 4 == 0:
                    x0g = cpost.tile([P, 4, D], HDT, name="x0g", bufs=1)
                    nc.sync.dma_start(
                        x0g[:], x0_d.ap().rearrange(
                            "(j p) d -> p j d", p=P)[:, j:j + 4, :])
                layer_norm(x0g[:, j % 4, :], x_f32[:, j, :], "ln0")
                if j > 0:
                    set_x_from(j - 1, ps_init)
            set_x_from(NT - 1, ps_init)
        if dbg:
            nc.sync.dma_start(
                dbg_tensors["dbg_x0ln"].ap().rearrange("(j p) d -> p j d", p=P),
                x_f32[:])

        # ---- layers ----
        prev_tail = None
        for layer in range(n_layers):
            # Phase A c=0 (+ previous layer's E chains j=4..7, stage 2 one
            # chain behind stage 1).
            with tc.tile_pool(name=f"psA0_{layer}", bufs=4,
                              space="PSUM") as psA0, \
                 tc.tile_pool(name=f"psE2_{layer}", bufs=2,
                              space="PSUM") as psE2:
                tail_ymg = None
                pa0 = PhaseA(0, psA0)
                if prev_tail is not None:
                    tail_ymg = load_ym_group(prev_tail, 4)
                for g in range(4):
                    pa0.positions(range(8 * g, 8 * g + 8))
                    if prev_tail is not None:
                        if g > 0:
                            chain_E2(3 + g, psE2, None)
                        chain_E1(tail_ymg, 4 + g, psE2)
                if prev_tail is not None:
                    chain_E2(7, psE2, None)
            prev_tail = None

            # Phase B c=0 interleaved with phase A c=1: the c=0 score chunks
            # only read qr columns 0:512.  Between chunks, a quarter of the
            # phase-A c=1 m-loop runs so its relu/rope stream stays ahead of
            # the c=1 score chunks that follow.  Each ykv accumulator is
            # scaled and DMA'd out to the AllReduce buffer the moment its
            # last matmul stops, so the collective fires right after the
            # final chunk instead of after a bulk copy.
            ykv_pre = arp.tile([P, NT, D], HDT, name="ykv_pre")
            ar_in = dram.tile([P, NT * D], HDT, name=f"arin_{layer}",
                              tag=f"arin_{layer}")

            def emit_ykv(ykv_ps, c, i, sc, sd, base):
                for j in range(max(4 * c, i), 4 * c + 4):
                    lhsT = sd[:] if j == i else \
                        sc[:, j * P - base:(j + 1) * P - base]
                    nc.tensor.matmul(
                        ykv_ps[j - 4 * c][:], lhsT=lhsT,
                        rhs=x_h[:, i, :],
                        start=(i == 0), stop=(i == j))
                if i >= 4 * c:
                    nc.scalar.mul(out=ykv_pre[:, i, :],
                                  in_=ykv_ps[i - 4 * c][:], mul=YKV_SCALE)
                    nc.sync.dma_start(ar_in[:, i * D:(i + 1) * D],
                                      ykv_pre[:, i, :])

            # The first two c=0 chunks contract their first NKA k-tiles in
            # an open pass so the PE never waits for the rope-c0 tail; the
            # remaining tiles land in pass b once the late pairs exist.
            with tc.tile_pool(name=f"psS0_{layer}", bufs=2,
                              space="PSUM") as psS0, \
                 tc.tile_pool(name=f"psY0_{layer}", bufs=1,
                              space="PSUM") as psY0, \
                 tc.tile_pool(name=f"psA1_{layer}", bufs=2,
                              space="PSUM") as psA1:
                ykv_ps = [psY0.tile([P, D], F32, name=f"ykv_ps{j}",
                                    tag=f"ykv_ps{j}") for j in range(4)]
                pa1 = PhaseA(1, psA1)
                pend = None
                for g in range(4):
                    args = s_chunk(0, g, psS0)
                    if pend is not None:
                        emit_ykv(ykv_ps, 0, *pend)
                    pend = args
                    pa1.positions(range(8 * g, 8 * g + 8))
                emit_ykv(ykv_ps, 0, *pend)

            # Phase B c=1: all eight score chunks, ykv one chunk behind.
            with tc.tile_pool(name=f"psS1_{layer}", bufs=4,
                              space="PSUM") as psS1, \
                 tc.tile_pool(name=f"psY1_{layer}", bufs=1,
                              space="PSUM") as psY1:
                ykv_ps = [psY1.tile([P, D], F32, name=f"ykv_ps{j}",
                                    tag=f"ykv_ps{j}") for j in range(4, 8)]
                pend = None
                for i in range(8):
                    args = s_chunk(1, i, psS1)
                    if pend is not None:
                        emit_ykv(ykv_ps, 1, *pend)
                    pend = args
                emit_ykv(ykv_ps, 1, *pend)

            if dbg and layer == 0:
                nc.sync.dma_start(
                    dbg_tensors["dbg_xsp"].ap().rearrange(
                        "(m p) t -> p m t", p=P), x_sp[:])
                nc.sync.dma_start(
                    dbg_tensors["dbg_qr"].ap().rearrange(
                        "(m p) t -> p m t", p=P), qr[:])
                nc.sync.dma_start(
                    dbg_tensors["dbg_ykvpre"].ap().rearrange(
                        "(j p) d -> p j d", p=P), ykv_pre[:])

            # Phase C: pair AllReduce of ykv; per-t-tile LN+transpose chains
            # two-staged (stage 1 has no PE work, so all loads+LNs fire
            # first and the transposes stream without per-chain stalls);
            # j=4..7 overlapped with phase D's c=0 half.
            ar_out = dram.tile([P, NT * D], HDT, name=f"arout_{layer}",
                               tag=f"arout_{layer}")
            emit_allreduce(nc, PAIR_GROUPS, [ar_in.opt()], [ar_out.opt()])

            ymlpT_pre = arp.tile([P, KD, T], HDT, name="ymlpT_pre")
            ar2_in = dram.tile([P, KD * T], HDT, name=f"ar2in_{layer}",
                               tag=f"ar2in_{layer}")
            with tc.tile_pool(name=f"psT_{layer}", bufs=2,
                              space="PSUM") as psT:
                ypg = load_yp_group(ar_out, 0)
                yls = [chain_C1(ypg, j) for j in range(4)]
                for j in range(4):
                    chain_C2(j, yls[j], psT)
                with tc.tile_pool(name=f"psD_{layer}", bufs=2,
                                  space="PSUM") as psD, \
                     tc.tile_pool(name=f"psM_{layer}", bufs=1,
                                  space="PSUM") as psM:
                    ymlpT_ps = [psM.tile([P, T], F32, name=f"ymlpT_ps{k}",
                                         tag=f"ymlpT_ps{k}")
                                for k in range(KD)]

                    def emit_dec(cols, m, wvt, half, xy):
                        for k in range(KD):
                            nc.tensor.matmul(
                                ymlpT_ps[k][:, cols],
                                lhsT=wvt[:, half,
                                         ETILE + k * P:ETILE + (k + 1) * P],
                                rhs=xy[:],
                                start=(m == 0), stop=(m == NM - 1))

                    for c in range(2):
                        cols = slice(c * TH, (c + 1) * TH)
                        pend = []
                        wvt = None
                        for m in range(NM):
                            if m % 2 == 0:
                                wvt = wdec.tile([P, 2, WTILE], HDT,
                                                name="wv_t")
                                nc.sync.dma_start(
                                    wvt[:],
                                    wv_d.ap()[:, m * WTILE:(m + 2) * WTILE])
                            half = m % 2
                            ps = psD.tile([P, TH], F32, name="psD")
                            for k in range(KD):
                                nc.tensor.matmul(
                                    ps[:],
                                    lhsT=wvt[:, half, k * P:(k + 1) * P],
                                    rhs=ykvT_h[:, k, cols],
                                    start=(k == 0), stop=(k == KD - 1))
                            ysp = yxp.tile([P, TH], HDT, name="ysp",
                                           bufs=2)
                            nc.scalar.activation(
                                out=ysp[:], in_=ps[:],
                                func=mybir.ActivationFunctionType.Relu)
                            xy = yxp.tile([P, TH], HDT, name="xy")
                            nc.vector.tensor_mul(xy[:], ysp[:],
                                                 x_sp[:, m, cols])
                            pend.append((m, wvt, half, xy))
                            if len(pend) > 1:
                                emit_dec(cols, *pend.pop(0))
                        for pd in pend:
                            emit_dec(cols, *pd)
                        if c == 0:
                            ypg = load_yp_group(ar_out, 4)
                            yls = [chain_C1(ypg, j) for j in range(4, NT)]
                            for j in range(4, NT):
                                chain_C2(j, yls[j - 4], psT)
                    for k in range(KD):
                        if k == 0:
                            nc.scalar.copy(out=ymlpT_pre[:, k, :],
                                           in_=ymlpT_ps[k][:])
                        else:
                            nc.vector.tensor_copy(out=ymlpT_pre[:, k, :],
                                                  in_=ymlpT_ps[k][:])
                        nc.sync.dma_start(ar2_in[:, k * T:(k + 1) * T],
                                          ymlpT_pre[:, k, :])

            if dbg and layer == 0:
                nc.sync.dma_start(
                    dbg_tensors["dbg_ykvT"].ap().rearrange(
                        "(k p) t -> p k t", p=P), ykvT_h[:])

            # Phase E: 8-way AllReduce of ymlp^T; x = ln(x + ymlp*rstd).
            # Chains j=0..3 here (stage 2 one chain behind stage 1);
            # j=4..7 ride inside the next layer's phase A c=0 (prev_tail).
            # Last layer emits everything plus logits inline.
            ar2_out = dram.tile([P, KD * T], HDT, name=f"ar2out_{layer}",
                                tag=f"ar2out_{layer}", addr_space="Shared")
            emit_allreduce(nc, ALL_GROUP, [ar2_in.opt()], [ar2_out.opt()])

            last = layer == n_layers - 1
            if last:
                psL = tc.alloc_tile_pool(name="psL", bufs=2, space="PSUM")
                with tc.tile_pool(name=f"psE_{layer}", bufs=2,
                                  space="PSUM") as psE:
                    for j in range(NT):
                        if j % 4 == 0:
                            ymg = load_ym_group(ar2_out, j)
                        chain_E1(ymg, j, psE)
                        if j > 0:
                            chain_E2(j - 1, psE, psL)
                    chain_E2(NT - 1, psE, psL)
                psL.release()
            else:
                with tc.tile_pool(name=f"psE_{layer}", bufs=2,
                                  space="PSUM") as psE:
                    ymg = load_ym_group(ar2_out, 0)
                    for j in range(4):
                        chain_E1(ymg, j, psE)
                        if j > 0:
                            chain_E2(j - 1, psE, None)
                    chain_E2(3, psE, None)

                prev_tail = ar2_out

            if dbg and layer == 0:
                nc.sync.dma_start(
                    dbg_tensors["dbg_x1"].ap().rearrange(
                        "(j p) d -> p j d", p=P), x_f32[:])

        for _pool in (statp, lnp, ylp, cpost, arp, yxp, sdp, schp, ropep,
                      csp, wdec, wenc, dram, persist):
            _pool.release()

    nc.compile()
    return nc


def _host_inputs(idx, embed, encoder, encoder_v, decoder, lm_head):
    """Build the 8 per-core input maps (host-side sharding + tile packing)."""
    f16 = np.float16
    idx = np.asarray(idx).reshape(-1).astype(np.int64)
    embed = np.asarray(embed, np.float32)
    enc = np.asarray(encoder, np.float32)
    encv = np.asarray(encoder_v, np.float32)
    dec = np.asarray(decoder, np.float32)
    lmh = np.asarray(lm_head, np.float32)

    x0 = embed[idx]  # [T, D] gather on host (pure indexing)

    # freqs exactly as the reference computes them (fp32)
    t = np.arange(0, N, dtype=np.float32)
    q = np.floor(t / 2.0) * 2.0
    freqs = (1.0 / ((2.0 ** 16) ** (q / N)) / TWO_PI).astype(np.float32)
    tvec = np.arange(T, dtype=np.float32)

    umask = (np.arange(P)[:, None] < np.arange(P)[None, :]).astype(np.float32)

    def tileize(w):
        """[D, NLOC] weight -> [P, NM, KD, P] partition-major tiles:
        out[p, m, k, n] = w[k*P + p, m*P + n]."""
        return np.ascontiguousarray(
            w.reshape(KD, P, NM, P).transpose(1, 2, 0, 3))

    in_maps = []
    for d in range(N_CORES):
        h, half = d // 2, d % 2
        perm = np.concatenate([np.arange(0, NLOC, 2),
                               np.arange(1, NLOC, 2)]) + half * NLOC
        f_loc = freqs[perm[:NLOC // 2]]
        ph = (tvec[None, :] * f_loc[:, None]).astype(np.float32) % 1.0
        ct = np.cos(TWO_PI * ph).astype(f16)   # [NPAIR*P, T]
        st = np.sin(TWO_PI * ph).astype(f16)
        # cos/sin fused per (pair, column-half): [P, NPAIR, 2, 2, TH]
        cst = np.stack([ct.reshape(NPAIR, P, 2, TH),
                        st.reshape(NPAIR, P, 2, TH)],
                       axis=3).transpose(1, 0, 2, 3, 4)

        enc_t = tileize(enc[h][:, perm].astype(f16))[:, PAIR_ORDER]
        encv_t = tileize(encv[h][:, perm].astype(f16))  # [P, NM, KD, P]
        # decoder tiles: [P, NM, D]; row p of tile m is latent m*P+p
        dec_t = dec[h * N + perm, :].astype(f16).reshape(NM, P, D)
        dec_t = np.ascontiguousarray(dec_t.transpose(1, 0, 2))
        # fused encv+dec per m: [P, NM, WTILE]
        wv = np.concatenate([encv_t.reshape(P, NM, ETILE), dec_t], axis=2)

        in_maps.append({
            "x0": np.ascontiguousarray(x0, f16),
            "enc_pk": np.ascontiguousarray(enc_t.reshape(P, NM * ETILE)),
            "wv_pk": np.ascontiguousarray(wv.reshape(P, NM * WTILE)),
            "cst_pk": np.ascontiguousarray(cst.reshape(P, NPAIR * 2 * T)),
            "lmh": np.ascontiguousarray(lmh, f16),
            "umask": umask,
        })
    return in_maps


def kernel(idx, embed, encoder, encoder_v, decoder, lm_head,
           _trace=False, _tmpdir=None):
    if "nc" not in _CACHE:
        _CACHE["nc"] = _build_program()
    nc = _CACHE["nc"]
    in_maps = _host_inputs(idx, embed, encoder, encoder_v, decoder, lm_head)
    res = bass_utils.run_bass_kernel_spmd(
        nc, in_maps, core_ids=list(range(N_CORES)),
        trace=_trace, tmpdir=_tmpdir)
    _CACHE["last_results"] = res
    logits = res.results[0]["logits"].astype(np.float32).reshape(B, T, VOCAB)
    return logits
